# revision 34
# baseline (speedup 1.0000x reference)
"""Causal self-attention (B=2, T=2048, C=1024, 16 heads) on 8 trn2 NeuronCores.

Sharding: tensor-parallel over heads (4-way) x data-parallel over batch (2-way).
Core r handles batch dp = r // 4 and heads [4*tp, 4*tp+4) where tp = r % 4.

Single globally-pipelined stream (vs the earlier 3-phase design): attention is
processed window-major (512-query windows), each window in two sequential
head-pair passes so the PV accumulators need only 2 PSUM banks.  QKV
projection matmuls for later windows and the out-projection matmuls for
earlier windows are fed into the attention chunk stream between chunks, so the
PE stays busy while the ACT engine works through the exp()s (the true
secondary bottleneck at ~82us of ACTIVATE work).  All PSUM drains (q/k bias,
v bias, out-proj copy, softmax normalize) run on the DVE/gpsimd so the ACT
engine does almost nothing but exp.

Per-pass attention machinery is unchanged from the baseline: S^T tiles = k q^T
with the two heads of a pair packed into disjoint PE row halves (concurrent
via tile_position), one [128,1024] exp covers both heads, causal masking via a
bf16 0/1 mask multiply on DVE, yhat^T = [v|1]^T P^T with the ones row giving
the softmax denominator, normalized via DVE reciprocal straight out of PSUM +
gpsimd partition broadcast.

The 4-way tensor-parallel reduction of the row-parallel projection is done on
the host over gathered fp16 partials (an in-kernel 4-core collective measures
150-340us on this axon setup -- more than the whole compute budget).
"""

import numpy as np

B, T, C = 2, 2048, 1024
NH, HD = 16, 64
NCORES, TPG = 8, 4          # 4-way tensor parallel x 2-way data parallel
HPC = NH // TPG             # heads per core (4)
DH = HPC * HD               # per-core head channels (256)
KC = C // 128               # contraction chunks over C (8)
NT4 = T // 512              # 512-wide q windows (4)
NT = T // 128               # 128-wide T tiles (16)
DEPTH = 3                   # PV stagger depth (chunks)

_PROG = None
TRACE = False
LAST_RESULTS = None


def _build():
    import concourse.bacc as bacc
    import concourse.mybir as mybir
    from concourse import tile

    F32 = mybir.dt.float32
    BF16 = mybir.dt.bfloat16
    F16 = mybir.dt.float16
    AF = mybir.ActivationFunctionType

    nc = bacc.Bacc("TRN2", target_bir_lowering=False, debug=False,
                   num_devices=NCORES)

    # all DRAM tensors are laid out partition-major on the host so every DMA
    # moves long per-partition contiguous runs (short descriptors were the
    # dominant input-stream cost)
    xT = nc.dram_tensor("xT", [128, KC, T], BF16, kind="ExternalInput").ap()
    wqT = nc.dram_tensor("wqT", [128, KC, DH], BF16, kind="ExternalInput").ap()
    wkT = nc.dram_tensor("wkT", [128, KC, DH], BF16, kind="ExternalInput").ap()
    wvT = nc.dram_tensor("wvT", [128, KC, DH], BF16, kind="ExternalInput").ap()
    wpT = nc.dram_tensor("wpT", [128, 2, C], BF16, kind="ExternalInput").ap()
    bq2 = nc.dram_tensor("bq2", [128, 2], F32, kind="ExternalInput").ap()
    bk2 = nc.dram_tensor("bk2", [128, 2], F32, kind="ExternalInput").ap()
    bv2 = nc.dram_tensor("bv2", [1, 512], F32, kind="ExternalInput").ap()
    koff_d = nc.dram_tensor("koff_d", [128, 4], F32, kind="ExternalInput").ap()
    qrow_d = nc.dram_tensor("qrow_d", [1, 1024], F32, kind="ExternalInput").ap()
    # yout[p, w, c, t'] = partial[c*128+p, 512*w + t']
    yout = nc.dram_tensor("yout", [128, NT4, KC, 512], F16, kind="ExternalOutput").ap()

    lp = nc.allow_low_precision

    with tile.TileContext(nc) as tc:
        with tc.tile_pool(name="const", bufs=1) as constp, \
             tc.tile_pool(name="data", bufs=1) as datap, \
             tc.tile_pool(name="strip", bufs=6) as stripp, \
             tc.tile_pool(name="norm", bufs=1) as normp, \
             tc.tile_pool(name="out", bufs=1) as outp, \
             tc.tile_pool(name="ps_pp", bufs=2, space="PSUM") as pp:
            # --- constants / weights ---
            wq_sb = constp.tile([128, KC, DH], BF16)
            wk_sb = constp.tile([128, KC, DH], BF16)
            wv_sb = constp.tile([128, KC, DH], BF16)
            wp_sb = constp.tile([128, 2, C], BF16)
            bq_sb = constp.tile([128, 2], F32)
            bk_sb = constp.tile([128, 2], F32)
            bv_sb = constp.tile([1, 512], F32)
            bv_bc = constp.tile([128, 512], F32)
            koff_sb = constp.tile([128, 4], F32)
            qrow_sb = constp.tile([1, 1024], F32)
            qrow_bc = constp.tile([128, 1024], F32)
            mask_sb = constp.tile([128, 4, 2, 512], BF16)

            # persistent activations
            xT_sb = datap.tile([128, KC, T], BF16)
            qT_sb = datap.tile([128, 2, T], BF16)   # [64*(h%2)+d, h//2, t]
            kT_sb = datap.tile([128, 2, T], BF16)
            v4 = datap.tile([128, NT, HPC, HD + 1], BF16)  # [t%128, t//128, h, d|1]
            yT_sb = datap.tile([128, 2, T], BF16)

            # ---- input DMA schedule (4 queues) ----
            # gpsimd DMA is software-managed (~4x slower) -- only small
            # tensors go there.  xT chunk-pairs alternate between the two
            # hardware queues; weights are ordered by first use.
            nc.sync.dma_start(out=wk_sb[:], in_=wkT[:])
            nc.sync.dma_start(out=xT_sb[:, 0:2, :], in_=xT[:, 0:2, :])
            nc.sync.dma_start(out=xT_sb[:, 4:6, :], in_=xT[:, 4:6, :])
            nc.sync.dma_start(out=wp_sb[:], in_=wpT[:])
            nc.scalar.dma_start(out=wq_sb[:], in_=wqT[:])
            nc.scalar.dma_start(out=wv_sb[:], in_=wvT[:])
            nc.scalar.dma_start(out=xT_sb[:, 2:4, :], in_=xT[:, 2:4, :])
            nc.scalar.dma_start(out=xT_sb[:, 6:8, :], in_=xT[:, 6:8, :])
            nc.gpsimd.dma_start(out=bq_sb[:], in_=bq2[:])
            nc.gpsimd.dma_start(out=bk_sb[:], in_=bk2[:])
            nc.gpsimd.dma_start(out=bv_sb[:], in_=bv2[:])
            nc.gpsimd.dma_start(out=koff_sb[:], in_=koff_d[:])
            nc.gpsimd.dma_start(out=qrow_sb[:], in_=qrow_d[:])
            nc.gpsimd.partition_broadcast(bv_bc[:], bv_sb[:])
            nc.gpsimd.partition_broadcast(qrow_bc[:], qrow_sb[:])
            # causal 0/1 mask and the v-ones column are generated on-device
            # (the 512KB mask DMA was pure prologue-critical-path waste)
            nc.vector.memset(v4[:, :, :, HD:HD + 1], 1.0)
            with lp(reason="0/1 mask gen"):
                for o in range(4):
                    nc.vector.tensor_scalar(
                        mask_sb[:, o, :, :],
                        qrow_bc[:].rearrange("p (h q) -> p h q", h=2),
                        koff_sb[:, o:o + 1], None,
                        mybir.AluOpType.is_ge)

            # ---- deferred projection work-units, fed between attention chunks
            pending = []  # FIFO of (deadline_window, fn)

            def qk_units(w, m, wsb, bsb, dst, dl):
                box = {}

                def mk(c):
                    def f():
                        if c == 0:
                            box["ps"] = pp.tile([128, 512], F32, tag="pp", name="ps")
                        nc.tensor.matmul(
                            box["ps"][:], lhsT=wsb[:, c, 128 * m:128 * (m + 1)],
                            rhs=xT_sb[:, c, 512 * w:512 * (w + 1)],
                            start=(c == 0), stop=(c == KC - 1))
                        if c == KC - 1:
                            with lp(reason="bf16 proj out"):
                                nc.vector.tensor_scalar_add(
                                    dst[:, m, 512 * w:512 * (w + 1)],
                                    box["ps"][:], bsb[:, m:m + 1])
                    return f
                return [(dl, mk(c)) for c in range(KC)]

            def v_units(w, half, dl):
                box = {}
                t0 = 4 * w + 2 * half

                def mk(c):
                    def f():
                        if c == 0:
                            box["ps"] = pp.tile([128, 512], F32, tag="pp", name="ps")
                        for dt_ in range(2):
                            # start=True clears the whole PSUM bank, so only
                            # the first MM into the shared bank may set it.
                            nc.tensor.matmul(
                                box["ps"][:, 256 * dt_:256 * (dt_ + 1)],
                                lhsT=xT_sb[:, c, 128 * (t0 + dt_):128 * (t0 + dt_ + 1)],
                                rhs=wv_sb[:, c, :],
                                start=(c == 0 and dt_ == 0),
                                stop=(c == KC - 1))
                        if c == KC - 1:
                            with lp(reason="bf16 v out"):
                                nc.vector.tensor_add(
                                    v4[:, t0:t0 + 2, :, 0:HD],
                                    box["ps"][:].rearrange("p (t h d) -> p t h d", t=2, h=HPC),
                                    bv_bc[:].rearrange("p (t h d) -> p t h d", t=2, h=HPC))
                    return f
                return [(dl, mk(c)) for c in range(KC)]

            def op_units(w, dl):
                box = {}

                def mk(mo):
                    def f():
                        if mo % 4 == 0:
                            box["ot"] = outp.tile([128, 4, 512], F16, tag="ot",
                                                  bufs=2, name="ot")
                        ps = pp.tile([128, 512], F32, tag="pp", name="ps")
                        for cc in range(2):
                            nc.tensor.matmul(
                                ps[:],
                                lhsT=wp_sb[:, cc, 128 * mo:128 * (mo + 1)],
                                rhs=yT_sb[:, cc, 512 * w:512 * (w + 1)],
                                start=(cc == 0), stop=(cc == 1))
                        # on the final window ACT is exp-free: give it half
                        # the PSUM->fp16 copies so DVE isn't the op3 choke
                        if w == NT4 - 1 and mo % 2 == 0:
                            nc.scalar.activation(box["ot"][:, mo % 4, :], ps[:],
                                                 AF.Copy)
                        else:
                            with lp(reason="fp16 partials"):
                                nc.vector.tensor_copy(box["ot"][:, mo % 4, :],
                                                      ps[:])
                        if mo % 4 == 3:
                            nc.sync.dma_start(
                                out=yout[:, w, mo - 3:mo + 1, :],
                                in_=box["ot"][:])
                    return f
                return [(dl, mk(mo)) for mo in range(KC)]

            def feed(k):
                n = 0
                while pending and n < k:
                    pending.pop(0)[1]()
                    n += 1

            def flush(dl):
                while pending and pending[0][0] <= dl:
                    pending.pop(0)[1]()

            # ---- prologue: qkv(w0), q(w1), k(w1-pre?) streamed against xT DMA
            # 6 extra PSUM banks live only before attention starts.
            with tc.tile_pool(name="pro", bufs=1, space="PSUM") as prop:
                pro_q = [[prop.tile([128, 512], F32, tag=f"q{w}{m}", name="pq")
                          for m in range(2)] for w in range(2)]
                pro_k = [prop.tile([128, 512], F32, tag=f"k0{m}", name="pk")
                         for m in range(2)]
                vbox = [pp.tile([128, 512], F32, tag="pp", name="ps")
                        for _ in range(2)]
                def pro_v(c):
                    # v MMs lag the q/k sweep by 2 chunks so a late-arriving
                    # wv never head-of-line-blocks the PE queue
                    for half in range(2):
                        t0 = 2 * half
                        for dt_ in range(2):
                            nc.tensor.matmul(
                                vbox[half][:, 256 * dt_:256 * (dt_ + 1)],
                                lhsT=xT_sb[:, c, 128 * (t0 + dt_):128 * (t0 + dt_ + 1)],
                                rhs=wv_sb[:, c, :],
                                start=(c == 0 and dt_ == 0),
                                stop=(c == KC - 1))

                for c in range(KC):
                    for w in range(2):
                        for m in range(2):
                            nc.tensor.matmul(
                                pro_q[w][m][:],
                                lhsT=wq_sb[:, c, 128 * m:128 * (m + 1)],
                                rhs=xT_sb[:, c, 512 * w:512 * (w + 1)],
                                start=(c == 0), stop=(c == KC - 1))
                    for m in range(2):
                        nc.tensor.matmul(
                            pro_k[m][:],
                            lhsT=wk_sb[:, c, 128 * m:128 * (m + 1)],
                            rhs=xT_sb[:, c, 0:512],
                            start=(c == 0), stop=(c == KC - 1))
                    if c >= 2:
                        pro_v(c - 2)
                for c in range(KC - 2, KC):
                    pro_v(c)
                with lp(reason="bf16 proj out"):
                    for w in range(2):
                        for m in range(2):
                            nc.vector.tensor_scalar_add(
                                qT_sb[:, m, 512 * w:512 * (w + 1)],
                                pro_q[w][m][:], bq_sb[:, m:m + 1])
                    for m in range(2):
                        nc.vector.tensor_scalar_add(
                            kT_sb[:, m, 0:512], pro_k[m][:], bk_sb[:, m:m + 1])
                    for half in range(2):
                        t0 = 2 * half
                        nc.vector.tensor_add(
                            v4[:, t0:t0 + 2, :, 0:HD],
                            vbox[half][:].rearrange("p (t h d) -> p t h d", t=2, h=HPC),
                            bv_bc[:].rearrange("p (t h d) -> p t h d", t=2, h=HPC))

            # remaining projection jobs for window 1 (q1 done in prologue)
            pending += qk_units(1, 0, wk_sb, bk_sb, kT_sb, dl=1)
            pending += qk_units(1, 1, wk_sb, bk_sb, kT_sb, dl=1)
            pending += v_units(1, 0, dl=1)
            pending += v_units(1, 1, dl=1)

            # ---- main pipeline: one flat chunk stream over (w, m, c).
            # The PV stagger and the pass-close normalization ride inside the
            # stream so the ACT engine never starves at pass boundaries.
            with tc.tile_pool(name="ps_s", bufs=2, space="PSUM") as ps_s, \
                 tc.tile_pool(name="ps_y", bufs=1, space="PSUM") as ps_y:

                def close_pass(w, m, psy):
                    # normalize psy -> yT.  gpsimd cannot read PSUM, so the
                    # denominator row is staged in SBUF (via ACT on the final
                    # pass, when it has no exps left; via DVE otherwise).
                    for hh in range(2):
                        den = normp.tile([1, 512], F32, tag=f"dn{hh}",
                                         bufs=4, name="den")
                        rrow = normp.tile([1, 512], F32, tag=f"rr{hh}",
                                          bufs=4, name="rrow")
                        rbc = normp.tile([64, 512], F32, tag=f"rb{hh}",
                                         bufs=4, name="rbc")
                        if w == NT4 - 1 and m == 1:
                            nc.scalar.activation(den[:], psy[hh][HD:HD + 1, :],
                                                 AF.Copy)
                        else:
                            nc.vector.tensor_copy(den[:], psy[hh][HD:HD + 1, :])
                        nc.vector.reciprocal_approx_fast(rrow[:], den[:])
                        nc.gpsimd.partition_broadcast(rbc[:], rrow[:])
                        with lp(reason="bf16 y out"):
                            nc.vector.tensor_mul(
                                yT_sb[64 * hh:64 * (hh + 1), m,
                                      512 * w:512 * (w + 1)],
                                psy[hh][0:HD, :], rbc[:])
                    # queue follow-on projection work as passes retire
                    if m == 1:
                        w2 = w + 2
                        if w2 < NT4:
                            pending.extend(
                                qk_units(w2, 0, wq_sb, bq_sb, qT_sb, dl=w2)
                                + qk_units(w2, 1, wq_sb, bq_sb, qT_sb, dl=w2)
                                + qk_units(w2, 0, wk_sb, bk_sb, kT_sb, dl=w2)
                                + qk_units(w2, 1, wk_sb, bk_sb, kT_sb, dl=w2)
                                + v_units(w2, 0, dl=w2)
                                + v_units(w2, 1, dl=w2))
                        pending.extend(op_units(w, dl=w + 1))

                stream = [(w, m, c) for w in range(NT4) for m in range(2)
                          for c in range(4 * (w + 1))]
                NCH = len(stream)
                state = {}
                pvq = []

                def mk_pv(w, m, c, strip, qo):
                    nch = 4 * (w + 1)

                    def f():
                        if c == 0:
                            state[(w, m)] = [
                                ps_y.tile([HD + 1, 512], F32, tag=f"psy{hh}",
                                          name="psy") for hh in range(2)]
                        psy = state[(w, m)]
                        for hh in range(2):
                            nc.tensor.matmul(
                                psy[hh][:, qo:],
                                lhsT=v4[:, c, 2 * m + hh, :],
                                rhs=strip[:, 512 * hh + qo:512 * (hh + 1)],
                                start=(c == 0), stop=(c == nch - 1))
                        if c == nch - 1:
                            close_pass(w, m, state.pop((w, m)))
                    return f

                for idx, (w, m, c) in enumerate(stream):
                    if c == 0 and m == 0:
                        flush(w)
                    o = c - 4 * w
                    qo = 128 * o if o > 0 else 0
                    pss2 = ps_s.tile([128, 1024], F32, tag="s", name="pss2")
                    for hh in range(2):
                        po = 64 * hh
                        nc.tensor.matmul(
                            pss2[:, 512 * hh + qo:512 * (hh + 1)],
                            lhsT=kT_sb[po:po + 64, m, 128 * c:128 * (c + 1)],
                            rhs=qT_sb[po:po + 64, m, 512 * w + qo:512 * (w + 1)],
                            start=True, stop=True, tile_position=(po, 0))
                    strip = stripp.tile([128, 1024], BF16, tag="stp",
                                        name="strip")
                    p3i = pss2[:].rearrange("p (h q) -> p h q", h=2)
                    p3o = strip[:].rearrange("p (h q) -> p h q", h=2)
                    nc.scalar.activation(p3o[:, :, qo:], p3i[:, :, qo:], AF.Exp)
                    if o >= 0:
                        with lp(reason="0/1 mask"):
                            nc.vector.tensor_mul(p3o[:, :, qo:], p3o[:, :, qo:],
                                                 mask_sb[:, o, :, qo:])
                    pvq.append(mk_pv(w, m, c, strip, qo))
                    keep = 1 if idx >= NCH - 3 else DEPTH
                    while len(pvq) > keep:
                        pvq.pop(0)()
                    if pending:
                        wleft = sum(1 for (w_, _, _) in stream[idx + 1:]
                                    if w_ == w) + 1
                        k = -(-len(pending) // wleft)
                        feed(min(k, 4))
                while pvq:
                    pvq.pop(0)()
                # dead matmuls hold the HAM clock gate warm across the final
                # close chain before the last out-projection burst
                warm = ps_s.tile([128, 1024], F32, tag="s", name="warm")
                for ww in range(12):
                    nc.tensor.matmul(warm[:, 0:512], lhsT=kT_sb[0:64, 0, 0:128],
                                     rhs=qT_sb[0:64, 0, 0:512],
                                     start=True, stop=True)
                flush(NT4)

    nc.compile()
    return nc


def _bf16():
    import ml_dtypes
    return ml_dtypes.bfloat16


def kernel(x, Wq, bq, Wk, bk, Wv, bv, Wp, bp):
    global _PROG, LAST_RESULTS
    from concourse.bass_utils import run_bass_kernel_spmd

    x = np.asarray(x, np.float32)
    Wq = np.asarray(Wq, np.float32)
    bq = np.asarray(bq, np.float32)
    Wk = np.asarray(Wk, np.float32)
    bk = np.asarray(bk, np.float32)
    Wv = np.asarray(Wv, np.float32)
    bv = np.asarray(bv, np.float32)
    Wp = np.asarray(Wp, np.float32)
    bp = np.asarray(bp, np.float32)

    if _PROG is None:
        _PROG = _build()
    nc = _PROG

    scale = np.float32(1.0 / np.sqrt(HD))
    in_maps = []
    for r in range(NCORES):
        tp, dp = r % TPG, r // TPG
        sl = slice(DH * tp, DH * (tp + 1))
        def pmaj(a, nchunk):     # [nchunk*128, F] -> [128, nchunk, F]
            return np.ascontiguousarray(
                a.reshape(nchunk, 128, a.shape[1]).transpose(1, 0, 2))
        koff = (np.arange(128)[:, None] + 128.0 * np.arange(4)[None, :])
        qrow = np.tile(np.arange(512.0), 2).reshape(1, 1024)
        in_maps.append({
            "xT": pmaj(x[dp].T, KC).astype(_bf16()),
            "wqT": pmaj((Wq[sl] * scale).T, KC).astype(_bf16()),
            "wkT": pmaj(Wk[sl].T, KC).astype(_bf16()),
            "wvT": pmaj(Wv[sl].T, KC).astype(_bf16()),
            "wpT": pmaj(Wp[:, sl].T, 2).astype(_bf16()),
            "bq2": np.ascontiguousarray((bq[sl] * scale).reshape(2, 128).T),
            "bk2": np.ascontiguousarray(bk[sl].reshape(2, 128).T),
            "bv2": np.tile(bv[sl], 2).reshape(1, 512).copy(),
            "koff_d": koff.astype(np.float32),
            "qrow_d": qrow.astype(np.float32),
        })

    res = run_bass_kernel_spmd(nc, in_maps, core_ids=list(range(NCORES)),
                               trace=TRACE)
    LAST_RESULTS = res

    out = np.empty((B, T, C), np.float32)
    for dp in range(B):
        acc = res.results[TPG * dp]["yout"].astype(np.float32)
        for tp in range(1, TPG):
            acc += res.results[TPG * dp + tp]["yout"].astype(np.float32)
        # yout[p, w, c, t'] = partial^T[c*128+p, 512*w+t'] -> out[t, co]
        out[dp] = acc.transpose(1, 3, 2, 0).reshape(T, C) + bp
    return out


# revision 39
# speedup vs baseline: 1.0474x; 1.0474x over previous
"""Causal self-attention (B=2, T=2048, C=1024, 16 heads) on 8 trn2 NeuronCores.

Sharding: tensor-parallel over heads (4-way) x data-parallel over batch (2-way).
Core r handles batch dp = r // 4 and heads [4*tp, 4*tp+4) where tp = r % 4.

Single globally-pipelined stream (vs the earlier 3-phase design): attention is
processed window-major (512-query windows), each window in two sequential
head-pair passes so the PV accumulators need only 2 PSUM banks.  QKV
projection matmuls for later windows and the out-projection matmuls for
earlier windows are fed into the attention chunk stream between chunks, so the
PE stays busy while the ACT engine works through the exp()s (the true
secondary bottleneck at ~82us of ACTIVATE work).  All PSUM drains (q/k bias,
v bias, out-proj copy, softmax normalize) run on the DVE/gpsimd so the ACT
engine does almost nothing but exp.

Per-pass attention machinery is unchanged from the baseline: S^T tiles = k q^T
with the two heads of a pair packed into disjoint PE row halves (concurrent
via tile_position), one [128,1024] exp covers both heads, causal masking via a
bf16 0/1 mask multiply on DVE, yhat^T = [v|1]^T P^T with the ones row giving
the softmax denominator, normalized via DVE reciprocal straight out of PSUM +
gpsimd partition broadcast.

The 4-way tensor-parallel reduction of the row-parallel projection is done on
the host over gathered fp16 partials (an in-kernel 4-core collective measures
150-340us on this axon setup -- more than the whole compute budget).
"""

import numpy as np

B, T, C = 2, 2048, 1024
NH, HD = 16, 64
NCORES, TPG = 8, 4          # 4-way tensor parallel x 2-way data parallel
HPC = NH // TPG             # heads per core (4)
DH = HPC * HD               # per-core head channels (256)
KC = C // 128               # contraction chunks over C (8)
NT4 = T // 512              # 512-wide q windows (4)
NT = T // 128               # 128-wide T tiles (16)
DEPTH = 3                   # PV stagger depth (chunks)

_PROG = None
TRACE = False
LAST_RESULTS = None


def _build():
    import concourse.bacc as bacc
    import concourse.mybir as mybir
    from concourse import tile

    F32 = mybir.dt.float32
    BF16 = mybir.dt.bfloat16
    F16 = mybir.dt.float16
    AF = mybir.ActivationFunctionType

    nc = bacc.Bacc("TRN2", target_bir_lowering=False, debug=False,
                   num_devices=NCORES)

    # all DRAM tensors are laid out partition-major on the host so every DMA
    # moves long per-partition contiguous runs (short descriptors were the
    # dominant input-stream cost)
    xT = nc.dram_tensor("xT", [128, KC, T], BF16, kind="ExternalInput").ap()
    wqT = nc.dram_tensor("wqT", [128, KC, DH], BF16, kind="ExternalInput").ap()
    wkT = nc.dram_tensor("wkT", [128, KC, DH], BF16, kind="ExternalInput").ap()
    wvT = nc.dram_tensor("wvT", [128, KC, DH], BF16, kind="ExternalInput").ap()
    wpT = nc.dram_tensor("wpT", [128, 2, C], BF16, kind="ExternalInput").ap()
    bq2 = nc.dram_tensor("bq2", [128, 2], F32, kind="ExternalInput").ap()
    bk2 = nc.dram_tensor("bk2", [128, 2], F32, kind="ExternalInput").ap()
    bv2 = nc.dram_tensor("bv2", [1, 512], F32, kind="ExternalInput").ap()
    koff_d = nc.dram_tensor("koff_d", [128, 4], F32, kind="ExternalInput").ap()
    qrow_d = nc.dram_tensor("qrow_d", [1, 1024], F32, kind="ExternalInput").ap()
    # yout[p, w, c, t'] = partial[c*128+p, 512*w + t']
    yout = nc.dram_tensor("yout", [128, NT4, KC, 512], F16, kind="ExternalOutput").ap()

    lp = nc.allow_low_precision

    with tile.TileContext(nc) as tc:
        with tc.tile_pool(name="const", bufs=1) as constp, \
             tc.tile_pool(name="data", bufs=1) as datap, \
             tc.tile_pool(name="strip", bufs=6) as stripp, \
             tc.tile_pool(name="norm", bufs=1) as normp, \
             tc.tile_pool(name="out", bufs=1) as outp, \
             tc.tile_pool(name="ps_pp", bufs=2, space="PSUM") as pp:
            # --- constants / weights ---
            wq_sb = constp.tile([128, KC, DH], BF16)
            wk_sb = constp.tile([128, KC, DH], BF16)
            wv_sb = constp.tile([128, KC, DH], BF16)
            wp_sb = constp.tile([128, 2, C], BF16)
            bq_sb = constp.tile([128, 2], F32)
            bk_sb = constp.tile([128, 2], F32)
            bv_sb = constp.tile([1, 512], F32)
            bv_bc = constp.tile([128, 512], F32)
            koff_sb = constp.tile([128, 4], F32)
            qrow_sb = constp.tile([1, 1024], F32)
            qrow_bc = constp.tile([128, 1024], F32)
            mask_sb = constp.tile([128, 4, 2, 512], BF16)

            # persistent activations
            xT_sb = datap.tile([128, KC, T], BF16)
            qT_sb = datap.tile([128, 2, T], BF16)   # [64*(h%2)+d, h//2, t]
            kT_sb = datap.tile([128, 2, T], BF16)
            v4 = datap.tile([128, NT, HPC, HD + 1], BF16)  # [t%128, t//128, h, d|1]
            yT_sb = datap.tile([128, 2, T], BF16)

            # ---- input DMA schedule (4 queues) ----
            # gpsimd DMA is software-managed (~4x slower) -- only small
            # tensors go there.  xT chunk-pairs alternate between the two
            # hardware queues; weights are ordered by first use.
            nc.sync.dma_start(out=wk_sb[:], in_=wkT[:])
            nc.sync.dma_start(out=xT_sb[:, 0:2, :], in_=xT[:, 0:2, :])
            nc.sync.dma_start(out=xT_sb[:, 4:6, :], in_=xT[:, 4:6, :])
            nc.scalar.dma_start(out=wq_sb[:], in_=wqT[:])
            nc.scalar.dma_start(out=wv_sb[:], in_=wvT[:])
            nc.scalar.dma_start(out=xT_sb[:, 2:4, :], in_=xT[:, 2:4, :])
            nc.scalar.dma_start(out=xT_sb[:, 6:8, :], in_=xT[:, 6:8, :])
            nc.gpsimd.dma_start(out=bq_sb[:], in_=bq2[:])
            nc.gpsimd.dma_start(out=bk_sb[:], in_=bk2[:])
            nc.gpsimd.dma_start(out=bv_sb[:], in_=bv2[:])
            nc.gpsimd.dma_start(out=koff_sb[:], in_=koff_d[:])
            nc.gpsimd.dma_start(out=qrow_sb[:], in_=qrow_d[:])
            nc.gpsimd.partition_broadcast(bv_bc[:], bv_sb[:])
            nc.gpsimd.partition_broadcast(qrow_bc[:], qrow_sb[:])
            # wp is needed only once out-projections start (deep into the
            # stream) -- park it on the slow gpsimd software-DMA queue to
            # keep prologue HW-queue bandwidth for xT and q/k/v weights
            nc.gpsimd.dma_start(out=wp_sb[:], in_=wpT[:])
            # causal 0/1 mask and the v-ones column are generated on-device
            # (the 512KB mask DMA was pure prologue-critical-path waste)
            nc.vector.memset(v4[:, :, :, HD:HD + 1], 1.0)
            with lp(reason="0/1 mask gen"):
                for o in range(4):
                    nc.vector.tensor_scalar(
                        mask_sb[:, o, :, :],
                        qrow_bc[:].rearrange("p (h q) -> p h q", h=2),
                        koff_sb[:, o:o + 1], None,
                        mybir.AluOpType.is_ge)

            # ---- deferred projection work-units, fed between attention chunks
            pending = []  # FIFO of (deadline_window, fn) -- deadline-gated qkv
            lazy = []     # out-proj units: deferred into the ACT-bound tail
                          # windows where the PE would otherwise idle

            def qk_units(w, m, wsb, bsb, dst, dl):
                box = {}

                def mk(c):
                    def f():
                        if c == 0:
                            box["ps"] = pp.tile([128, 512], F32, tag="pp", name="ps")
                        nc.tensor.matmul(
                            box["ps"][:], lhsT=wsb[:, c, 128 * m:128 * (m + 1)],
                            rhs=xT_sb[:, c, 512 * w:512 * (w + 1)],
                            start=(c == 0), stop=(c == KC - 1))
                        if c == KC - 1:
                            with lp(reason="bf16 proj out"):
                                nc.vector.tensor_scalar_add(
                                    dst[:, m, 512 * w:512 * (w + 1)],
                                    box["ps"][:], bsb[:, m:m + 1])
                    return f
                return [(dl, mk(c)) for c in range(KC)]

            def v_units(w, half, dl):
                box = {}
                t0 = 4 * w + 2 * half

                def mk(c):
                    def f():
                        if c == 0:
                            box["ps"] = pp.tile([128, 512], F32, tag="pp", name="ps")
                        for dt_ in range(2):
                            # start=True clears the whole PSUM bank, so only
                            # the first MM into the shared bank may set it.
                            nc.tensor.matmul(
                                box["ps"][:, 256 * dt_:256 * (dt_ + 1)],
                                lhsT=xT_sb[:, c, 128 * (t0 + dt_):128 * (t0 + dt_ + 1)],
                                rhs=wv_sb[:, c, :],
                                start=(c == 0 and dt_ == 0),
                                stop=(c == KC - 1))
                        if c == KC - 1:
                            with lp(reason="bf16 v out"):
                                nc.vector.tensor_add(
                                    v4[:, t0:t0 + 2, :, 0:HD],
                                    box["ps"][:].rearrange("p (t h d) -> p t h d", t=2, h=HPC),
                                    bv_bc[:].rearrange("p (t h d) -> p t h d", t=2, h=HPC))
                    return f
                return [(dl, mk(c)) for c in range(KC)]

            def op_units(w, dl):
                box = {}

                def mk(mo):
                    def f():
                        if mo % 4 == 0:
                            box["ot"] = outp.tile([128, 4, 512], F16, tag="ot",
                                                  bufs=2, name="ot")
                        ps = pp.tile([128, 512], F32, tag="pp", name="ps")
                        for cc in range(2):
                            nc.tensor.matmul(
                                ps[:],
                                lhsT=wp_sb[:, cc, 128 * mo:128 * (mo + 1)],
                                rhs=yT_sb[:, cc, 512 * w:512 * (w + 1)],
                                start=(cc == 0), stop=(cc == 1))
                        # on the final window ACT is exp-free: give it half
                        # the PSUM->fp16 copies so DVE isn't the op3 choke
                        if w == NT4 - 1 and mo % 2 == 0:
                            nc.scalar.activation(box["ot"][:, mo % 4, :], ps[:],
                                                 AF.Copy)
                        else:
                            with lp(reason="fp16 partials"):
                                nc.vector.tensor_copy(box["ot"][:, mo % 4, :],
                                                      ps[:])
                        if mo % 4 == 3:
                            nc.sync.dma_start(
                                out=yout[:, w, mo - 3:mo + 1, :],
                                in_=box["ot"][:])
                    return f
                return [(dl, mk(mo)) for mo in range(KC)]

            def feed(k):
                n = 0
                while pending and n < k:
                    pending.pop(0)[1]()
                    n += 1

            def flush(dl):
                while pending and pending[0][0] <= dl:
                    pending.pop(0)[1]()

            # ---- prologue: qkv(w0), q(w1), k(w1-pre?) streamed against xT DMA
            # 6 extra PSUM banks live only before attention starts.
            with tc.tile_pool(name="pro", bufs=1, space="PSUM") as prop:
                pro_q = [[prop.tile([128, 512], F32, tag=f"q{w}{m}", name="pq")
                          for m in range(2)] for w in range(2)]
                pro_k = [prop.tile([128, 512], F32, tag=f"k0{m}", name="pk")
                         for m in range(2)]
                vbox = [pp.tile([128, 512], F32, tag="pp", name="ps")
                        for _ in range(2)]
                def pro_v(c):
                    # v MMs lag the q/k sweep by 2 chunks so a late-arriving
                    # wv never head-of-line-blocks the PE queue
                    for half in range(2):
                        t0 = 2 * half
                        for dt_ in range(2):
                            nc.tensor.matmul(
                                vbox[half][:, 256 * dt_:256 * (dt_ + 1)],
                                lhsT=xT_sb[:, c, 128 * (t0 + dt_):128 * (t0 + dt_ + 1)],
                                rhs=wv_sb[:, c, :],
                                start=(c == 0 and dt_ == 0),
                                stop=(c == KC - 1))

                for c in range(KC):
                    for w in range(2):
                        for m in range(2):
                            nc.tensor.matmul(
                                pro_q[w][m][:],
                                lhsT=wq_sb[:, c, 128 * m:128 * (m + 1)],
                                rhs=xT_sb[:, c, 512 * w:512 * (w + 1)],
                                start=(c == 0), stop=(c == KC - 1))
                    for m in range(2):
                        nc.tensor.matmul(
                            pro_k[m][:],
                            lhsT=wk_sb[:, c, 128 * m:128 * (m + 1)],
                            rhs=xT_sb[:, c, 0:512],
                            start=(c == 0), stop=(c == KC - 1))
                    if c >= 2:
                        pro_v(c - 2)
                for c in range(KC - 2, KC):
                    pro_v(c)
                with lp(reason="bf16 proj out"):
                    for w in range(2):
                        for m in range(2):
                            nc.vector.tensor_scalar_add(
                                qT_sb[:, m, 512 * w:512 * (w + 1)],
                                pro_q[w][m][:], bq_sb[:, m:m + 1])
                    for m in range(2):
                        nc.vector.tensor_scalar_add(
                            kT_sb[:, m, 0:512], pro_k[m][:], bk_sb[:, m:m + 1])
                    for half in range(2):
                        t0 = 2 * half
                        nc.vector.tensor_add(
                            v4[:, t0:t0 + 2, :, 0:HD],
                            vbox[half][:].rearrange("p (t h d) -> p t h d", t=2, h=HPC),
                            bv_bc[:].rearrange("p (t h d) -> p t h d", t=2, h=HPC))

            # remaining projection jobs for window 1 (q1 done in prologue)
            pending += qk_units(1, 0, wk_sb, bk_sb, kT_sb, dl=1)
            pending += qk_units(1, 1, wk_sb, bk_sb, kT_sb, dl=1)
            pending += v_units(1, 0, dl=1)
            pending += v_units(1, 1, dl=1)

            # ---- main pipeline: one flat chunk stream over (w, m, c).
            # The PV stagger and the pass-close normalization ride inside the
            # stream so the ACT engine never starves at pass boundaries.
            with tc.tile_pool(name="ps_s", bufs=2, space="PSUM") as ps_s, \
                 tc.tile_pool(name="ps_y", bufs=1, space="PSUM") as ps_y:

                def close_pass(w, m, psy):
                    # normalize psy -> yT.  gpsimd cannot read PSUM, so the
                    # denominator row is staged in SBUF (via ACT on the final
                    # pass, when it has no exps left; via DVE otherwise).
                    for hh in range(2):
                        den = normp.tile([1, 512], F32, tag=f"dn{hh}",
                                         bufs=4, name="den")
                        rrow = normp.tile([1, 512], F32, tag=f"rr{hh}",
                                          bufs=4, name="rrow")
                        rbc = normp.tile([64, 512], F32, tag=f"rb{hh}",
                                         bufs=4, name="rbc")
                        if w == NT4 - 1 and m == 1:
                            nc.scalar.activation(den[:], psy[hh][HD:HD + 1, :],
                                                 AF.Copy)
                        else:
                            nc.vector.tensor_copy(den[:], psy[hh][HD:HD + 1, :])
                        nc.vector.reciprocal_approx_fast(rrow[:], den[:])
                        nc.gpsimd.partition_broadcast(rbc[:], rrow[:])
                        with lp(reason="bf16 y out"):
                            nc.vector.tensor_mul(
                                yT_sb[64 * hh:64 * (hh + 1), m,
                                      512 * w:512 * (w + 1)],
                                psy[hh][0:HD, :], rbc[:])
                    # queue follow-on projection work as passes retire
                    if m == 1:
                        w2 = w + 2
                        if w2 < NT4:
                            pending.extend(
                                qk_units(w2, 0, wq_sb, bq_sb, qT_sb, dl=w2)
                                + qk_units(w2, 1, wq_sb, bq_sb, qT_sb, dl=w2)
                                + qk_units(w2, 0, wk_sb, bk_sb, kT_sb, dl=w2)
                                + qk_units(w2, 1, wk_sb, bk_sb, kT_sb, dl=w2)
                                + v_units(w2, 0, dl=w2)
                                + v_units(w2, 1, dl=w2))
                        lazy.extend(op_units(w, dl=NT4))

                stream = [(w, m, c) for w in range(NT4) for m in range(2)
                          for c in range(4 * (w + 1))]
                NCH = len(stream)
                state = {}
                pvq = []

                def mk_pv(w, m, c, strip, qo):
                    nch = 4 * (w + 1)

                    def f():
                        if c == 0:
                            state[(w, m)] = [
                                ps_y.tile([HD + 1, 512], F32, tag=f"psy{hh}",
                                          name="psy") for hh in range(2)]
                        psy = state[(w, m)]
                        for hh in range(2):
                            nc.tensor.matmul(
                                psy[hh][:, qo:],
                                lhsT=v4[:, c, 2 * m + hh, :],
                                rhs=strip[:, 512 * hh + qo:512 * (hh + 1)],
                                start=(c == 0), stop=(c == nch - 1))
                        if c == nch - 1:
                            close_pass(w, m, state.pop((w, m)))
                    return f

                for idx, (w, m, c) in enumerate(stream):
                    if c == 0 and m == 0:
                        flush(w)
                    o = c - 4 * w
                    qo = 128 * o if o > 0 else 0
                    pss2 = ps_s.tile([128, 1024], F32, tag="s", name="pss2")
                    for hh in range(2):
                        po = 64 * hh
                        nc.tensor.matmul(
                            pss2[:, 512 * hh + qo:512 * (hh + 1)],
                            lhsT=kT_sb[po:po + 64, m, 128 * c:128 * (c + 1)],
                            rhs=qT_sb[po:po + 64, m, 512 * w + qo:512 * (w + 1)],
                            start=True, stop=True, tile_position=(po, 0))
                    strip = stripp.tile([128, 1024], BF16, tag="stp",
                                        name="strip")
                    p3i = pss2[:].rearrange("p (h q) -> p h q", h=2)
                    p3o = strip[:].rearrange("p (h q) -> p h q", h=2)
                    nc.scalar.activation(p3o[:, :, qo:], p3i[:, :, qo:], AF.Exp)
                    if o >= 0:
                        with lp(reason="0/1 mask"):
                            nc.vector.tensor_mul(p3o[:, :, qo:], p3o[:, :, qo:],
                                                 mask_sb[:, o, :, qo:])
                    pvq.append(mk_pv(w, m, c, strip, qo))
                    keep = 1 if idx >= NCH - 3 else DEPTH
                    while len(pvq) > keep:
                        pvq.pop(0)()
                    if pending:
                        wleft = sum(1 for (w_, _, _) in stream[idx + 1:]
                                    if w_ == w) + 1
                        k = -(-len(pending) // wleft)
                        feed(min(k, 4))
                    elif lazy and w >= 2:
                        left = NCH - idx
                        k = -(-len(lazy) // left)
                        for _ in range(min(k, 2)):
                            if lazy:
                                lazy.pop(0)[1]()
                while pvq:
                    pvq.pop(0)()
                # dead matmuls hold the HAM clock gate warm across the final
                # close chain before the last out-projection burst
                warm = ps_s.tile([128, 1024], F32, tag="s", name="warm")
                for ww in range(12):
                    nc.tensor.matmul(warm[:, 0:512], lhsT=kT_sb[0:64, 0, 0:128],
                                     rhs=qT_sb[0:64, 0, 0:512],
                                     start=True, stop=True)
                flush(NT4)
                while lazy:
                    lazy.pop(0)[1]()

    nc.compile()
    return nc


def _bf16():
    import ml_dtypes
    return ml_dtypes.bfloat16


def kernel(x, Wq, bq, Wk, bk, Wv, bv, Wp, bp):
    global _PROG, LAST_RESULTS
    from concourse.bass_utils import run_bass_kernel_spmd

    x = np.asarray(x, np.float32)
    Wq = np.asarray(Wq, np.float32)
    bq = np.asarray(bq, np.float32)
    Wk = np.asarray(Wk, np.float32)
    bk = np.asarray(bk, np.float32)
    Wv = np.asarray(Wv, np.float32)
    bv = np.asarray(bv, np.float32)
    Wp = np.asarray(Wp, np.float32)
    bp = np.asarray(bp, np.float32)

    if _PROG is None:
        _PROG = _build()
    nc = _PROG

    scale = np.float32(1.0 / np.sqrt(HD))
    in_maps = []
    for r in range(NCORES):
        tp, dp = r % TPG, r // TPG
        sl = slice(DH * tp, DH * (tp + 1))
        def pmaj(a, nchunk):     # [nchunk*128, F] -> [128, nchunk, F]
            return np.ascontiguousarray(
                a.reshape(nchunk, 128, a.shape[1]).transpose(1, 0, 2))
        koff = (np.arange(128)[:, None] + 128.0 * np.arange(4)[None, :])
        qrow = np.tile(np.arange(512.0), 2).reshape(1, 1024)
        in_maps.append({
            "xT": pmaj(x[dp].T, KC).astype(_bf16()),
            "wqT": pmaj((Wq[sl] * scale).T, KC).astype(_bf16()),
            "wkT": pmaj(Wk[sl].T, KC).astype(_bf16()),
            "wvT": pmaj(Wv[sl].T, KC).astype(_bf16()),
            "wpT": pmaj(Wp[:, sl].T, 2).astype(_bf16()),
            "bq2": np.ascontiguousarray((bq[sl] * scale).reshape(2, 128).T),
            "bk2": np.ascontiguousarray(bk[sl].reshape(2, 128).T),
            "bv2": np.tile(bv[sl], 2).reshape(1, 512).copy(),
            "koff_d": koff.astype(np.float32),
            "qrow_d": qrow.astype(np.float32),
        })

    res = run_bass_kernel_spmd(nc, in_maps, core_ids=list(range(NCORES)),
                               trace=TRACE)
    LAST_RESULTS = res

    out = np.empty((B, T, C), np.float32)
    for dp in range(B):
        acc = res.results[TPG * dp]["yout"].astype(np.float32)
        for tp in range(1, TPG):
            acc += res.results[TPG * dp + tp]["yout"].astype(np.float32)
        # yout[p, w, c, t'] = partial^T[c*128+p, 512*w+t'] -> out[t, co]
        out[dp] = acc.transpose(1, 3, 2, 0).reshape(T, C) + bp
    return out
